# revision 16
# baseline (speedup 1.0000x reference)
"""Trainium2 Bass kernel for nn_EnhancedDLinear (8-core SPMD, full I/O).

Mathematical reductions (verified against the jax reference):

1. ``LayerNorm(1)`` normalizes a size-1 axis, so its output is the constant
   ``ln_b``; the entire detail branch (convs, adaptive softmax, [N,S,S]
   attention) collapses to a host-precomputed constant row ``dp_row``.
2. The replicate-pad moving average (k=25) is a linear map folded into the
   first trend/seasonal MLP layers -> one [336, 336] weight ``w1``.
3. The channel-mean feeding the fusion MLP is computed by appending
   column-sum columns to the second-layer weights; per-output biases ride
   constant-1 rows appended to the contraction (no broadcast DMAs).

Sharding: batch b -> core b (B=8, 8 cores), zero collectives.

Layout (per core, all big operands bf16: 1 cycle/row at any moving dim):
- L1: h1T[u,c] = relu(w1.T @ x + b1): 3 K-chunks x 3 M-tiles, psum [112,96].
- L2 splits into a tiny column-sum group (unblocks the fusion-softmax chain
  early) and the tp/sp main group which overlaps the softmax chain on PE.
- Fusion softmax: z1 -> relu -> z2[1,288] -> Exp(+accum den) -> recip;
  PE-transposes of the exp row run concurrently, then one tensor_scalar
  (PTR recip) normalizes fw columns.
- G = fw_t*tp + fw_s*sp + fw_d*dp built with two fused scalar_tensor_tensor
  ops; final projection rides constant-1 rows for both output biases.
"""

import numpy as np
import ml_dtypes

import concourse.bacc as bacc
import concourse.tile as tile
from concourse import mybir
from concourse.bass_utils import run_bass_kernel_spmd

B, S, C, P = 8, 336, 96, 96
HID = 168
MAIN_K = 25
N_CORES = 8
KC = 112          # K chunk (336 = 3*112)

BF16 = ml_dtypes.bfloat16
_CACHE = {}


def _mavg_matrix(s, k):
    # mt = xc @ Mm for the replicate-padded moving average
    p = (k - 1) // 2
    m = np.zeros((s, s), np.float64)
    for j in range(s):
        for d in range(-p, p + 1):
            i = min(max(j + d, 0), s - 1)
            m[i, j] += 1.0 / k
    return m.astype(np.float32)


def _build_module():
    f32 = mybir.dt.float32
    bf = mybir.dt.bfloat16
    nc = bacc.Bacc("TRN2", target_bir_lowering=False, debug=False,
                   num_devices=N_CORES)

    xb = nc.dram_tensor("xb", [KC, 288], bf, kind="ExternalInput")
    wa = nc.dram_tensor("wa", [KC, 1008], bf, kind="ExternalInput")
    w2p = nc.dram_tensor("w2p", [113, 582], bf, kind="ExternalInput")
    zpk = nc.dram_tensor("zpk", [96, 352], bf, kind="ExternalInput")
    opk = nc.dram_tensor("opk", [96, 240], bf, kind="ExternalInput")
    cf = nc.dram_tensor("cf", [KC, 104], f32, kind="ExternalInput")
    y = nc.dram_tensor("y", [P, P], f32, kind="ExternalOutput")

    AF = mybir.ActivationFunctionType
    OP = mybir.AluOpType

    with tile.TileContext(nc) as tc:
        with (
            tc.tile_pool(name="wp", bufs=1) as wp,
            tc.tile_pool(name="hp", bufs=1) as hp,
            tc.tile_pool(name="pp", bufs=8, space="PSUM") as pp,
        ):
            xbs = wp.tile([KC, 288], bf, tag="xbs")
            was = wp.tile([KC, 1008], bf, tag="was")
            w2s = wp.tile([113, 582], bf, tag="w2s")
            zps = wp.tile([96, 352], bf, tag="zps")
            ops = wp.tile([96, 240], bf, tag="ops")
            cfs = wp.tile([KC, 104], f32, tag="cfs")

            # constant-1 rows for bias folding: whole-tile memsets (engines
            # can't address partition bases like 112); the later ACT writes
            # overwrite rows 0:N, leaving the constant-1 bias row intact
            h1c = [hp.tile([113 if j == 0 else KC, 96], bf,
                           tag=f"h1c_{j}", name=f"h1c_{j}")
                   for j in range(3)]
            z1s = hp.tile([33, 1], bf, tag="z1s")
            hs = hp.tile([49, 96], bf, tag="hs")
            nc.vector.memset(h1c[0], 1.0)
            nc.vector.memset(z1s, 1.0)
            nc.vector.memset(hs, 1.0)

            nc.sync.dma_start(out=xbs, in_=xb[:, :])
            nc.scalar.dma_start(out=was, in_=wa[:, :])
            nc.gpsimd.dma_start(out=cfs, in_=cf[:, :])
            nc.gpsimd.dma_start(out=w2s, in_=w2p[:, :])
            nc.sync.dma_start(out=zps, in_=zpk[:, :])
            nc.scalar.dma_start(out=ops, in_=opk[:, :])

            # ---- L1: h1T[u, c] = relu(w1.T @ x + b1), bf16 out ----
            for m in range(3):
                ps = pp.tile([KC, 96], f32, tag="ps")
                for j in range(3):
                    nc.tensor.matmul(
                        ps, was[:, 336 * j + KC * m:336 * j + KC * (m + 1)],
                        xbs[:, 96 * j:96 * (j + 1)],
                        start=(j == 0), stop=(j == 2))
                nc.scalar.activation(h1c[m][0:KC, :], ps, AF.Relu,
                                     bias=cfs[0:KC, m:m + 1])

            # ---- ts2 (column sums) first: unblocks the softmax chain ----
            ps_ts = pp.tile([96, 2], f32, tag="ps")
            nc.tensor.matmul(ps_ts, h1c[0], w2s[:, 576:578],
                             start=True, stop=False)
            nc.tensor.matmul(ps_ts, h1c[1], w2s[0:KC, 578:580],
                             start=False, stop=False)
            nc.tensor.matmul(ps_ts, h1c[2], w2s[0:KC, 580:582],
                             start=False, stop=True)
            ts2b = hp.tile([96, 2], bf, tag="ts2b")
            nc.scalar.activation(ts2b, ps_ts, AF.Copy)

            # ---- L2 main: block-diagonal [tp | sp] (overlaps softmax chain)
            ps_tpsp = pp.tile([96, 192], f32, tag="ps", name="ps_tpsp")
            nc.tensor.matmul(ps_tpsp, h1c[0], w2s[:, 0:192],
                             start=True, stop=False)
            nc.tensor.matmul(ps_tpsp, h1c[1], w2s[0:KC, 192:384],
                             start=False, stop=False)
            nc.tensor.matmul(ps_tpsp, h1c[2], w2s[0:KC, 384:576],
                             start=False, stop=True)
            ps_tp = ps_tpsp[:, 0:96]
            ps_sp = ps_tpsp[:, 96:192]

            # ---- fusion softmax chain ----
            ps_z1 = pp.tile([32, 1], f32, tag="ps")
            nc.tensor.matmul(ps_z1, zps[:, 288:320], ts2b[:, 0:1],
                             start=True, stop=False)
            nc.tensor.matmul(ps_z1, zps[:, 320:352], ts2b[:, 1:2],
                             start=False, stop=True)
            nc.scalar.activation(z1s[0:32, :], ps_z1, AF.Relu,
                                 bias=cfs[0:32, 3:4])

            ps_z2 = pp.tile([1, 288], f32, tag="ps")
            nc.tensor.matmul(ps_z2, z1s, zps[0:33, 0:288],
                             start=True, stop=True)
            e_row = hp.tile([1, 288], f32, tag="e_row")
            den = hp.tile([1, 1], f32, tag="den")
            nc.scalar.activation(e_row, ps_z2, AF.Exp, accum_out=den)
            # broadcast den to 96 partitions via ones-row matmul, then
            # per-partition reciprocal
            ps_dc = pp.tile([96, 1], f32, tag="ps", name="ps_dc")
            nc.tensor.matmul(ps_dc, cfs[0:1, 8:104], den,
                             start=True, stop=True)
            recip = hp.tile([96, 1], f32, tag="recip")
            nc.vector.reciprocal(recip, ps_dc)

            # transpose exp chunks into per-partition columns (runs on PE
            # concurrently with the den/recip path)
            onef = cfs[0:1, 5:6]
            ps_fw = pp.tile([96, 3], f32, tag="ps", name="ps_fw")
            for k in range(3):
                nc.tensor.matmul(ps_fw[:, k:k + 1],
                                 e_row[0:1, 96 * k:96 * (k + 1)], onef,
                                 is_transpose=True, skip_group_check=True)
            fwc = hp.tile([96, 3], f32, tag="fwc")
            nc.vector.tensor_scalar_mul(fwc, ps_fw, recip)

            # ---- G = fw_t*tp + fw_s*sp + fw_d*dp  (c on partitions) ----
            gd = hp.tile([96, 96], f32, tag="gd")
            nc.vector.tensor_scalar_mul(gd, ops[:, 48:144], fwc[:, 2:3])
            gs = hp.tile([96, 96], f32, tag="gs")
            nc.vector.scalar_tensor_tensor(gs, ps_sp, fwc[:, 1:2], gd,
                                           OP.mult, OP.add)
            g = hp.tile([96, 96], bf, tag="g")
            nc.vector.scalar_tensor_tensor(g, ps_tp, fwc[:, 0:1], gs,
                                           OP.mult, OP.add)

            # ---- final projection ----
            ps_h = pp.tile([48, 96], f32, tag="ps")
            nc.tensor.matmul(ps_h, ops[:, 0:48], g, start=True, stop=True)
            nc.scalar.activation(hs[0:48, :], ps_h, AF.Relu,
                                 bias=cfs[0:48, 4:5])
            ps_o = pp.tile([96, 96], f32, tag="ps")
            nc.tensor.matmul(ps_o, hs, ops[0:49, 144:240],
                             start=True, stop=True)
            out_s = hp.tile([96, 96], f32, tag="out")
            nc.vector.tensor_copy(out_s, ps_o)
            nc.sync.dma_start(out=y[:, :], in_=out_s)

    nc.compile()
    return nc


def _prep_weights(i):
    f = np.float32
    mm = _mavg_matrix(S, MAIN_K)
    w1 = np.empty((S, 2 * HID), f)
    w1[:, :HID] = mm @ i['lt1w'].T.astype(f)
    w1[:, HID:] = (np.eye(S, dtype=f) - mm) @ i['ls1w'].T.astype(f)
    wa = np.empty((KC, 1008), BF16)
    for j in range(3):
        wa[:, 336 * j:336 * (j + 1)] = w1[KC * j:KC * (j + 1), :].astype(BF16)

    # constant detail_pred row (LayerNorm(1) output == ln_b exactly)
    xf = np.full((S,), f(i['ln_b'][0]), f)
    dp_row = (np.maximum(xf @ i['op1w'].T + i['op1b'], 0)
              @ i['op2w'].T + i['op2b']).astype(f)
    b1f = (i['fn1b']
           + dp_row.mean(dtype=f) * i['fn1w'][:, 2 * C:].sum(1)).astype(f)

    lt2wt = np.ascontiguousarray(i['lt2w'].T, f)   # [168, 96]
    ls2wt = np.ascontiguousarray(i['ls2w'].T, f)
    lt2b = i['lt2b'].astype(f)
    ls2b = i['ls2b'].astype(f)
    lt2s = lt2wt.sum(1)
    ls2s = ls2wt.sum(1)

    # block-diagonal [tp | sp] second-layer weights; chunk-0's constant-1
    # row (113th) carries both biases and their sums
    w2p = np.zeros((113, 582), f)
    w2p[0:112, 0:96] = lt2wt[0:112]
    w2p[112, 0:96] = lt2b
    w2p[112, 96:192] = ls2b
    w2p[0:56, 192:288] = lt2wt[112:168]
    w2p[56:112, 288:384] = ls2wt[0:56]
    w2p[0:112, 480:576] = ls2wt[56:168]
    w2p[0:112, 576] = lt2s[0:112]
    w2p[112, 576] = lt2b.sum(dtype=f)
    w2p[112, 577] = ls2b.sum(dtype=f)
    w2p[0:56, 578] = lt2s[112:168]
    w2p[56:112, 579] = ls2s[0:56]
    w2p[0:112, 581] = ls2s[56:168]

    zpk = np.zeros((96, 352), f)
    zpk[0:32, 0:288] = i['fn2w'].T
    zpk[32, 0:288] = i['fn2b']
    zpk[0:96, 288:320] = i['fn1w'][:, 0:C].T / C
    zpk[0:96, 320:352] = i['fn1w'][:, C:2 * C].T / C

    opk = np.zeros((96, 240), f)
    opk[0:96, 0:48] = i['fp1w'].T
    opk[0:96, 48:144] = np.tile(dp_row[None, :], (96, 1))
    opk[0:48, 144:240] = i['fp2w'].T
    opk[48, 144:240] = i['fp2b']

    cf = np.zeros((KC, 104), f)
    b1 = np.concatenate([i['lt1b'], i['ls1b']]).astype(f)
    for j in range(3):
        cf[0:KC, j] = b1[KC * j:KC * (j + 1)]
    cf[0:32, 3] = b1f
    cf[0:48, 4] = i['fp1b']
    cf[0, 5] = 1.0
    cf[0, 8:104] = 1.0

    return dict(wa=wa, w2p=w2p.astype(BF16), zpk=zpk.astype(BF16),
                opk=opk.astype(BF16), cf=cf)


def make_in_maps(inputs):
    shared = _prep_weights(inputs)
    x = np.asarray(inputs['x'], np.float32)
    in_maps = []
    for b in range(N_CORES):
        xbp = np.empty((KC, 288), BF16)
        for j in range(3):
            xbp[:, 96 * j:96 * (j + 1)] = x[b, KC * j:KC * (j + 1), :].astype(BF16)
        in_maps.append(dict(shared, xb=xbp))
    return in_maps


def kernel(**inputs):
    if "nc" not in _CACHE:
        _CACHE["nc"] = _build_module()
    res = run_bass_kernel_spmd(_CACHE["nc"], make_in_maps(inputs),
                               core_ids=list(range(N_CORES)))
    return np.stack([res.results[b]["y"] for b in range(N_CORES)], 0)


# revision 21
# speedup vs baseline: 1.4747x; 1.4747x over previous
"""Trainium2 Bass kernel for nn_EnhancedDLinear (8-core SPMD, full I/O).

Mathematical reductions (verified against the jax reference):

1. ``LayerNorm(1)`` normalizes a size-1 axis, so its output is the constant
   ``ln_b``; the entire detail branch (convs, adaptive softmax, [N,S,S]
   attention) collapses to a host-precomputed constant row ``dp_row``.
2. The replicate-pad moving average (k=25) is a linear map folded into the
   first trend/seasonal MLP layers -> one [336, 336] weight ``w1``.
3. The channel-mean feeding the fusion MLP is computed by appending
   column-sum columns to the second-layer weights; per-output biases ride
   constant-1 rows appended to the contraction (no broadcast DMAs).

Sharding: batch b -> core b (B=8, 8 cores), zero collectives.

Layout (per core, all big operands bf16: 1 cycle/row at any moving dim):
- L1: h1T[u,c] = relu(w1.T @ x + b1): 3 K-chunks x 3 M-tiles, psum [112,96].
- L2 splits into a tiny column-sum group (unblocks the fusion-softmax chain
  early) and the tp/sp main group which overlaps the softmax chain on PE.
- Fusion softmax: z1 -> relu -> z2[1,288] -> Exp(+accum den) -> recip;
  PE-transposes of the exp row run concurrently, then one tensor_scalar
  (PTR recip) normalizes fw columns.
- G = fw_t*tp + fw_s*sp + fw_d*dp built with two fused scalar_tensor_tensor
  ops; final projection rides constant-1 rows for both output biases.
"""

import numpy as np
import ml_dtypes

import concourse.bacc as bacc
import concourse.tile as tile
from concourse import mybir
from concourse.bass_utils import run_bass_kernel_spmd

B, S, C, P = 8, 336, 96, 96
HID = 168
MAIN_K = 25
N_CORES = 8
KC = 112          # K chunk (336 = 3*112)

BF16 = ml_dtypes.bfloat16
_CACHE = {}


def _mavg_matrix(s, k):
    # mt = xc @ Mm for the replicate-padded moving average
    p = (k - 1) // 2
    m = np.zeros((s, s), np.float64)
    for j in range(s):
        for d in range(-p, p + 1):
            i = min(max(j + d, 0), s - 1)
            m[i, j] += 1.0 / k
    return m.astype(np.float32)


def _build_module():
    f32 = mybir.dt.float32
    bf = mybir.dt.bfloat16
    nc = bacc.Bacc("TRN2", target_bir_lowering=False, debug=False,
                   num_devices=N_CORES)

    xb = nc.dram_tensor("xb", [KC, 288], bf, kind="ExternalInput")
    wa = nc.dram_tensor("wa", [KC, 1008], bf, kind="ExternalInput")
    w2p = nc.dram_tensor("w2p", [113, 582], bf, kind="ExternalInput")
    zpk = nc.dram_tensor("zpk", [96, 352], bf, kind="ExternalInput")
    opk = nc.dram_tensor("opk", [96, 240], bf, kind="ExternalInput")
    cf = nc.dram_tensor("cf", [KC, 8], f32, kind="ExternalInput")
    y = nc.dram_tensor("y", [P, P], f32, kind="ExternalOutput")

    AF = mybir.ActivationFunctionType
    OP = mybir.AluOpType

    with tile.TileContext(nc) as tc:
        with (
            tc.tile_pool(name="wp", bufs=1) as wp,
            tc.tile_pool(name="hp", bufs=1) as hp,
            tc.tile_pool(name="pp", bufs=8, space="PSUM") as pp,
        ):
            xbs = wp.tile([KC, 288], bf, tag="xbs")
            was = wp.tile([KC, 1008], bf, tag="was")
            w2s = wp.tile([113, 582], bf, tag="w2s")
            zps = wp.tile([96, 352], bf, tag="zps")
            ops = wp.tile([96, 240], bf, tag="ops")
            cfs = wp.tile([KC, 8], f32, tag="cfs")

            # constant-1 rows for bias folding: whole-tile memsets (engines
            # can't address partition bases like 112); the later ACT writes
            # overwrite rows 0:N, leaving the constant-1 bias row intact
            h1c = [hp.tile([113 if j == 0 else KC, 96], bf,
                           tag=f"h1c_{j}", name=f"h1c_{j}")
                   for j in range(3)]
            z1s = hp.tile([33, 1], bf, tag="z1s")
            hs = hp.tile([49, 96], bf, tag="hs")
            ones = hp.tile([1, 96], f32, tag="ones")
            nc.vector.memset(h1c[0], 1.0)
            nc.vector.memset(z1s, 1.0)
            nc.vector.memset(hs, 1.0)
            nc.vector.memset(ones, 1.0)

            # big transfers split across the two HWDGE queues (per-queue DMA
            # bandwidth is the bottleneck); the w2p bias row goes separately
            # (a 113-partition transfer degrades to one SDMA engine)
            nc.sync.dma_start(out=xbs, in_=xb[:, :])
            nc.scalar.dma_start(out=was[:, 0:504], in_=wa[:, 0:504])
            nc.sync.dma_start(out=was[:, 504:1008], in_=wa[:, 504:1008])
            nc.gpsimd.dma_start(out=cfs, in_=cf[:, :])
            nc.scalar.dma_start(out=w2s[0:112, :], in_=w2p[0:112, :])
            nc.scalar.dma_start(out=w2s[112:113, :], in_=w2p[112:113, :])
            nc.sync.dma_start(out=zps, in_=zpk[:, :])
            nc.gpsimd.dma_start(out=ops, in_=opk[:, :])

            # ---- L1: h1T[u, c] = relu(w1.T @ x + b1), bf16 out ----
            for m in range(3):
                ps = pp.tile([KC, 96], f32, tag="ps")
                for j in range(3):
                    nc.tensor.matmul(
                        ps, was[:, 336 * j + KC * m:336 * j + KC * (m + 1)],
                        xbs[:, 96 * j:96 * (j + 1)],
                        start=(j == 0), stop=(j == 2))
                nc.scalar.activation(h1c[m][0:KC, :], ps, AF.Relu,
                                     bias=cfs[0:KC, m:m + 1])

            # ---- ts2 (column sums) first: unblocks the softmax chain ----
            ps_ts = pp.tile([96, 2], f32, tag="ps")
            nc.tensor.matmul(ps_ts, h1c[0], w2s[:, 576:578],
                             start=True, stop=False)
            nc.tensor.matmul(ps_ts, h1c[1], w2s[0:KC, 578:580],
                             start=False, stop=False)
            nc.tensor.matmul(ps_ts, h1c[2], w2s[0:KC, 580:582],
                             start=False, stop=True)
            ts2b = hp.tile([96, 2], bf, tag="ts2b")
            nc.scalar.activation(ts2b, ps_ts, AF.Copy)

            # ---- L2 main: block-diagonal [tp | sp] (overlaps softmax chain)
            ps_tpsp = pp.tile([96, 192], f32, tag="ps", name="ps_tpsp")
            nc.tensor.matmul(ps_tpsp, h1c[0], w2s[:, 0:192],
                             start=True, stop=False)
            nc.tensor.matmul(ps_tpsp, h1c[1], w2s[0:KC, 192:384],
                             start=False, stop=False)
            nc.tensor.matmul(ps_tpsp, h1c[2], w2s[0:KC, 384:576],
                             start=False, stop=True)
            ps_tp = ps_tpsp[:, 0:96]
            ps_sp = ps_tpsp[:, 96:192]

            # ---- fusion softmax chain ----
            ps_z1 = pp.tile([32, 1], f32, tag="ps")
            nc.tensor.matmul(ps_z1, zps[:, 288:320], ts2b[:, 0:1],
                             start=True, stop=False)
            nc.tensor.matmul(ps_z1, zps[:, 320:352], ts2b[:, 1:2],
                             start=False, stop=True)
            nc.scalar.activation(z1s[0:32, :], ps_z1, AF.Relu,
                                 bias=cfs[0:32, 3:4])

            ps_z2 = pp.tile([1, 288], f32, tag="ps")
            nc.tensor.matmul(ps_z2, z1s, zps[0:33, 0:288],
                             start=True, stop=True)
            e_row = hp.tile([1, 288], f32, tag="e_row")
            den = hp.tile([1, 1], f32, tag="den")
            nc.scalar.activation(e_row, ps_z2, AF.Exp, accum_out=den)

            # transpose exp chunks into per-partition columns; the fusion
            # weights stay UNNORMALIZED here — 1/den is folded into the
            # final Relu's per-partition scale, so the den-broadcast matmul
            # and reciprocal run off the critical path in parallel with the
            # G-combination below
            onef = cfs[0:1, 5:6]
            ps_fw = pp.tile([96, 3], f32, tag="ps", name="ps_fw")
            for k in range(3):
                nc.tensor.matmul(ps_fw[:, k:k + 1],
                                 e_row[0:1, 96 * k:96 * (k + 1)], onef,
                                 is_transpose=True, skip_group_check=True)
            ps_dc = pp.tile([96, 1], f32, tag="ps", name="ps_dc")
            nc.tensor.matmul(ps_dc, ones, den, start=True, stop=True)
            recip = hp.tile([96, 1], f32, tag="recip")
            nc.vector.reciprocal(recip, ps_dc)

            # ---- G' = den*(fw_t*tp + fw_s*sp + fw_d*dp), c on partitions ----
            gd = hp.tile([96, 96], f32, tag="gd")
            nc.vector.tensor_scalar_mul(gd, ops[:, 48:144], ps_fw[:, 2:3])
            gs = hp.tile([96, 96], f32, tag="gs")
            nc.vector.scalar_tensor_tensor(gs, ps_sp, ps_fw[:, 1:2], gd,
                                           OP.mult, OP.add)
            g = hp.tile([96, 96], bf, tag="g")
            nc.vector.scalar_tensor_tensor(g, ps_tp, ps_fw[:, 0:1], gs,
                                           OP.mult, OP.add)

            # ---- final projection (Relu scale renormalizes by 1/den) ----
            ps_h = pp.tile([48, 96], f32, tag="ps")
            nc.tensor.matmul(ps_h, ops[:, 0:48], g, start=True, stop=True)
            nc.scalar.activation(hs[0:48, :], ps_h, AF.Relu,
                                 bias=cfs[0:48, 4:5], scale=recip[0:48, 0:1])
            ps_o = pp.tile([96, 96], f32, tag="ps")
            nc.tensor.matmul(ps_o, hs, ops[0:49, 144:240],
                             start=True, stop=True)
            out_s = hp.tile([96, 96], f32, tag="out")
            nc.vector.tensor_copy(out_s, ps_o)
            nc.sync.dma_start(out=y[:, :], in_=out_s)

    nc.compile()
    return nc


def _prep_weights(i):
    f = np.float32
    mm = _mavg_matrix(S, MAIN_K)
    w1 = np.empty((S, 2 * HID), f)
    w1[:, :HID] = mm @ i['lt1w'].T.astype(f)
    w1[:, HID:] = (np.eye(S, dtype=f) - mm) @ i['ls1w'].T.astype(f)
    wa = np.empty((KC, 1008), BF16)
    for j in range(3):
        wa[:, 336 * j:336 * (j + 1)] = w1[KC * j:KC * (j + 1), :].astype(BF16)

    # constant detail_pred row (LayerNorm(1) output == ln_b exactly)
    xf = np.full((S,), f(i['ln_b'][0]), f)
    dp_row = (np.maximum(xf @ i['op1w'].T + i['op1b'], 0)
              @ i['op2w'].T + i['op2b']).astype(f)
    b1f = (i['fn1b']
           + dp_row.mean(dtype=f) * i['fn1w'][:, 2 * C:].sum(1)).astype(f)

    lt2wt = np.ascontiguousarray(i['lt2w'].T, f)   # [168, 96]
    ls2wt = np.ascontiguousarray(i['ls2w'].T, f)
    lt2b = i['lt2b'].astype(f)
    ls2b = i['ls2b'].astype(f)
    lt2s = lt2wt.sum(1)
    ls2s = ls2wt.sum(1)

    # block-diagonal [tp | sp] second-layer weights; chunk-0's constant-1
    # row (113th) carries both biases and their sums
    w2p = np.zeros((113, 582), f)
    w2p[0:112, 0:96] = lt2wt[0:112]
    w2p[112, 0:96] = lt2b
    w2p[112, 96:192] = ls2b
    w2p[0:56, 192:288] = lt2wt[112:168]
    w2p[56:112, 288:384] = ls2wt[0:56]
    w2p[0:112, 480:576] = ls2wt[56:168]
    w2p[0:112, 576] = lt2s[0:112]
    w2p[112, 576] = lt2b.sum(dtype=f)
    w2p[112, 577] = ls2b.sum(dtype=f)
    w2p[0:56, 578] = lt2s[112:168]
    w2p[56:112, 579] = ls2s[0:56]
    w2p[0:112, 581] = ls2s[56:168]

    zpk = np.zeros((96, 352), f)
    zpk[0:32, 0:288] = i['fn2w'].T
    zpk[32, 0:288] = i['fn2b']
    zpk[0:96, 288:320] = i['fn1w'][:, 0:C].T / C
    zpk[0:96, 320:352] = i['fn1w'][:, C:2 * C].T / C

    opk = np.zeros((96, 240), f)
    opk[0:96, 0:48] = i['fp1w'].T
    opk[0:96, 48:144] = np.tile(dp_row[None, :], (96, 1))
    opk[0:48, 144:240] = i['fp2w'].T
    opk[48, 144:240] = i['fp2b']

    cf = np.zeros((KC, 8), f)
    b1 = np.concatenate([i['lt1b'], i['ls1b']]).astype(f)
    for j in range(3):
        cf[0:KC, j] = b1[KC * j:KC * (j + 1)]
    cf[0:32, 3] = b1f
    cf[0:48, 4] = i['fp1b']
    cf[0, 5] = 1.0

    return dict(wa=wa, w2p=w2p.astype(BF16), zpk=zpk.astype(BF16),
                opk=opk.astype(BF16), cf=cf)


def make_in_maps(inputs):
    shared = _prep_weights(inputs)
    x = np.asarray(inputs['x'], np.float32)
    in_maps = []
    for b in range(N_CORES):
        xbp = np.empty((KC, 288), BF16)
        for j in range(3):
            xbp[:, 96 * j:96 * (j + 1)] = x[b, KC * j:KC * (j + 1), :].astype(BF16)
        in_maps.append(dict(shared, xb=xbp))
    return in_maps


def kernel(**inputs):
    if "nc" not in _CACHE:
        _CACHE["nc"] = _build_module()
    res = run_bass_kernel_spmd(_CACHE["nc"], make_in_maps(inputs),
                               core_ids=list(range(N_CORES)))
    return np.stack([res.results[b]["y"] for b in range(N_CORES)], 0)
